# revision 29
# baseline (speedup 1.0000x reference)
"""GATEncoder kernel for 8 Trainium2 NeuronCores.

Strategy (hardcoded for the nn_GATEncoder problem):
  - Only nodes < batch_size (8192) reach the output; aggregation/decoder run
    for 8192 target nodes, sharded 1024 per core (8 windows of 128).
  - The GAT projection W_g is linear, so messages aggregate in h2-space
    (256 wide) and W_g applies per-head AFTER aggregation:
        agg_h[t] = W_gh @ (sum_e alpha_{e,h} h2[src_e]).
    The per-edge gather payload is a 384-bf16 row [h2(256) | 1 | a_src(2) |
    pad]; the ones column makes the agg matmul produce the softmax
    normalizer z as column 256 for free.
  - Encoder (Linear-LN-ReLU-Linear) is sharded across the 8 cores (1280
    nodes each: the core's 1024 targets + 256 of the leftover nodes); each
    core writes its bf16 table shard node-major (computed directly via
    matmul with lhsT=activations - no transposes), then an AllGather
    assembles the full 10240x384 table in every core's DRAM.
  - Edges partitioned by target core, bucketed into 8 windows of 128
    targets, sorted by source within a window, padded to a uniform tile
    count.  Per 128-edge tile: dma_gather pulls rows; attention logits via
    one-hot matmuls (a_dst broadcast) in PSUM; exp/leaky on ACT/DVE; the
    weighted one-hot (both heads built in one DVE op) aggregates messages
    into per-window PSUM.
  - Epilogue: per-head transform W_g + skip GEMM accumulate in the same
    PSUM group, ELU, decoder (all bf16 matmuls, f32 accumulate).
"""

import math

import numpy as np
import ml_dtypes

N_NODES = 10000
NPAD = 10240
N_EDGES = 160000
N_IN, H, HEADS, HOUT = 128, 256, 2, 256
BATCH = 8192
NCORES = 8
TPC = BATCH // NCORES          # 1024 target nodes per core
P = 128
WPC = TPC // P                 # 8 windows per core
ROW = 384                      # table row (bf16): 256 h2 + 1 + 2 a_src + pad
SHARD = NPAD // NCORES         # 1280 nodes computed per core (sharded mode)
BN = 256                       # encoder block size (nodes)
MM = HEADS * HOUT              # 512
F32 = np.float32
BF16 = np.dtype(ml_dtypes.bfloat16)

SHARDED = True                 # encoder sharded + AllGather (else replicated)

_cache = {}


# ----------------------------------------------------------------------------
# Host-side preprocessing
# ----------------------------------------------------------------------------

def _table_pos():
    """Global node id -> table row (sharded layout)."""
    pos = np.empty(NPAD, dtype=np.int64)
    n = np.arange(NPAD)
    tgt = n < BATCH
    pos[tgt] = (n[tgt] // TPC) * SHARD + n[tgt] % TPC
    m = n[~tgt] - BATCH
    pos[~tgt] = (m // (SHARD - TPC)) * SHARD + TPC + m % (SHARD - TPC)
    return pos


def _core_rows(c):
    """Global node ids for core c's shard, in local (table) order."""
    t = np.arange(c * TPC, (c + 1) * TPC, dtype=np.int64)
    e0 = BATCH + c * (SHARD - TPC)
    e = np.arange(e0, e0 + (SHARD - TPC), dtype=np.int64)
    return np.concatenate([t, e])


def _prepare_edges(edge_index):
    src = np.asarray(edge_index[0], dtype=np.int64)
    tgt = np.asarray(edge_index[1], dtype=np.int64)
    loops = np.arange(N_NODES, dtype=np.int64)
    src = np.concatenate([src, loops])
    tgt = np.concatenate([tgt, loops])
    keep = tgt < BATCH
    src, tgt = src[keep], tgt[keep]

    core = tgt // TPC
    tloc = tgt - core * TPC
    win = tloc // P
    trel = tloc - win * P

    buckets = {}
    counts = np.zeros((NCORES, WPC), dtype=np.int64)
    for c in range(NCORES):
        m = core == c
        sc, wc, rc = src[m], win[m], trel[m]
        for w in range(WPC):
            mw = wc == w
            s, r = sc[mw], rc[mw]
            o = np.argsort(s, kind="stable")
            buckets[(c, w)] = (s[o], r[o])
            counts[c, w] = s.size

    tiles_per_win = [int(math.ceil(counts[:, w].max() / P)) for w in range(WPC)]
    tiles_per_win = [max(t, 1) for t in tiles_per_win]
    return buckets, tiles_per_win


def _per_core_arrays(buckets, tiles_per_win, c, pos):
    """(gidx int16 wrapped, meta bf16 [P,TILES], ohT u8 [P,TILES,P])."""
    ntiles = sum(tiles_per_win)
    srcs = np.zeros(ntiles * P, dtype=np.int64)      # pad slots gather row 0
    trel = np.full(ntiles * P, -1.0, dtype=F32)      # -1 -> contributes 0
    t0 = 0
    for w in range(WPC):
        s, r = buckets[(c, w)]
        n = s.size
        base = t0 * P
        srcs[base : base + n] = s
        trel[base : base + n] = r.astype(F32)
        t0 += tiles_per_win[w]

    gidx = pos[srcs].astype(np.int16)
    tot = gidx.size
    wrapped = gidx.reshape(tot // 16, 16).T          # [16, tot/16]
    wrapped = np.tile(wrapped, (8, 1)).copy()        # [128, tot/16]

    meta = trel.reshape(ntiles, P).T.astype(BF16).copy()   # [P, TILES]

    # one-hot, DMA-friendly layout [j, tile, p] (contiguous per j,tile run)
    tr = trel.reshape(ntiles, P)
    iota = np.arange(P, dtype=F32)
    ohT = (tr[None, :, :] == iota[:, None, None]).astype(BF16)
    return wrapped, meta, np.ascontiguousarray(ohT)


# ----------------------------------------------------------------------------
# Bass program
# ----------------------------------------------------------------------------

def _build_program(tiles_per_win, sharded=SHARDED):
    import concourse.bacc as bacc
    import concourse.mybir as mybir
    import concourse.tile as tile

    dt = mybir.dt
    Alu = mybir.AluOpType
    Act = mybir.ActivationFunctionType

    TILES = sum(tiles_per_win)
    NLOC = SHARD if sharded else NPAD
    NBLK = NLOC // BN

    nc = bacc.Bacc("TRN2", target_bir_lowering=False)

    def inp(name, shape, dtype=dt.float32):
        return nc.dram_tensor(name, shape, dtype, kind="ExternalInput")

    # packed constant blobs (one DMA each); column layouts must match
    # _pack_consts() on the host side.
    CBF = 256 + 256 + 522 + 1024 + 1024 + 4096 + 8 + 256 + 128 + TILES
    CF32 = 261 + 2 + 2 + 2 + 2 + 4 + 8 + 1 + 1
    xT = inp("xT", [P, NLOC], dt.bfloat16)
    cbf_in = inp("cbf", [P, CBF], dt.bfloat16)
    cf32_in = inp("cf32", [P, CF32])
    gidx_in = inp("gidx", [P, (TILES * P) // 16], dt.int16)
    ohT_in = inp("ohT", [P, TILES, P], dt.bfloat16)

    y_out = nc.dram_tensor("y", [1, TPC], dt.float32, kind="ExternalOutput")

    GH = 8                      # tiles per gather call

    with tile.TileContext(nc) as tc:
        with (
            tc.tile_pool(name="const", bufs=1) as cpool,
            tc.tile_pool(name="persist", bufs=1) as ppool,
            tc.tile_pool(name="dram", bufs=1, space="DRAM") as dpool,
        ):
            # ---- packed constants / weights to SBUF (one DMA per blob) ----
            cbf = cpool.tile([P, CBF], dt.bfloat16, name="cbf", tag="cbf")
            nc.sync.dma_start(out=cbf[:], in_=cbf_in[:])
            cf32 = cpool.tile([P, CF32], dt.float32, name="cf32", tag="cf32")
            nc.sync.dma_start(out=cf32[:], in_=cf32_in[:])
            gidx_s = cpool.tile([P, (TILES * P) // 16], dt.int16,
                                name="gidx", tag="gidx")
            nc.sync.dma_start(out=gidx_s[:], in_=gidx_in[:])

            def _slicer(tile):
                state = {"off": 0}

                def take(shape):
                    n = int(np.prod(shape[1:]))
                    o = state["off"]
                    state["off"] += n
                    s = tile[:, o : o + n]
                    if len(shape) == 3:
                        s = s.rearrange("p (k c) -> p k c", k=shape[1])
                    return s
                take.state = state
                return take

            bfs = _slicer(cbf)
            w1s = bfs([P, H])
            negpos = bfs([P, 2, P])
            w2a = bfs([P, 2, 261])
            gats = bfs([P, 2, MM])
            skips = bfs([P, 2, MM])
            d1s = bfs([P, 4, 4 * H])
            d2s = bfs([P, 8, 1])
            iota2 = bfs([P, 2, P])
            ident = bfs([P, P])
            meta_s = bfs([P, TILES])
            assert bfs.state["off"] == CBF

            f32 = _slicer(cf32)
            b2rep = f32([P, 261])
            b2c = f32([P, 2, 1])
            b1s = f32([P, 2, 1])
            lng = f32([P, 2, 1])
            lnb = f32([P, 2, 1])
            gbsk = f32([P, 4, 1])
            db1s = f32([P, 8, 1])
            db2s = f32([P, 1])
            ln01 = f32([P, 1])
            assert f32.state["off"] == CF32

            if sharded:
                cc_in = dpool.tile([SHARD, ROW], dt.bfloat16, name="cc_in",
                                   tag="cc_in")
                cc_out = dpool.tile([NPAD, ROW], dt.bfloat16, name="cc_out",
                                    tag="cc_out", addr_space="Shared")
                tab_w, tab_r = cc_in, cc_out
            else:
                T_tab = dpool.tile([NPAD, ROW], dt.bfloat16, name="T_tab",
                                   tag="T_tab")
                tab_w = tab_r = T_tab

            # persistent across phases
            h2loc = ppool.tile([P, 2, TPC], dt.bfloat16, name="h2loc",
                               tag="h2loc")
            adstw = ppool.tile([P, WPC, 2], dt.bfloat16, name="adstw",
                               tag="adstw")
            aggs = ppool.tile([P, WPC, 2, HOUT], dt.bfloat16, name="aggs",
                              tag="aggs")

            # ================= Phase A: encoder -> table shard =============
            with (
                tc.tile_pool(name="wA", bufs=3) as wA,
                tc.tile_pool(name="psA", bufs=2, space="PSUM") as psA,
                tc.tile_pool(name="psA1", bufs=2, space="PSUM") as psA1,
            ):
                for b in range(NBLK):
                    n0 = b * BN
                    xb = wA.tile([P, BN], dt.bfloat16, name="xb", tag="xb")
                    nc.sync.dma_start(out=xb[:], in_=xT[:, n0 : n0 + BN])

                    h1 = wA.tile([P, 2, BN], dt.bfloat16, name="h1", tag="h1")
                    for m in range(2):
                        h1ps = psA.tile([P, BN], dt.float32, name="h1ps",
                                        tag="h1ps")
                        nc.tensor.matmul(
                            h1ps[:], lhsT=w1s[:, m * P : (m + 1) * P],
                            rhs=xb[:], start=True, stop=True)
                        nc.scalar.activation(h1[:, m, :], h1ps[:],
                                             Act.Identity,
                                             bias=b1s[:, m, 0:1])
                    sq = wA.tile([P, 2, BN], dt.bfloat16, name="sq", tag="sq")
                    nc.vector.tensor_mul(sq[:], h1[:], h1[:])

                    # stats: -mean and E[x^2] (separate banks - start=True
                    # zeroes the whole 2KB PSUM region)
                    mups = psA1.tile([P, BN], dt.float32, name="mups",
                                     tag="mups")
                    ex2ps = psA1.tile([P, BN], dt.float32, name="ex2ps",
                                      tag="ex2ps")
                    for m in range(2):
                        nc.tensor.matmul(mups[:], lhsT=negpos[:, 0, :],
                                         rhs=h1[:, m, :],
                                         start=(m == 0), stop=(m == 1))
                    for m in range(2):
                        nc.tensor.matmul(ex2ps[:], lhsT=negpos[:, 1, :],
                                         rhs=sq[:, m, :],
                                         start=(m == 0), stop=(m == 1))
                    musq = wA.tile([P, BN], dt.float32, name="musq",
                                   tag="musq")
                    nc.scalar.activation(musq[:], mups[:], Act.Square)
                    var = wA.tile([P, BN], dt.float32, name="var", tag="var")
                    nc.vector.scalar_tensor_tensor(
                        var[:], ex2ps[:], 1e-5, musq[:],
                        op0=Alu.add, op1=Alu.subtract)
                    rv = wA.tile([P, BN], dt.float32, name="rv", tag="rv")
                    nc.vector.reciprocal(rv[:], var[:])
                    rstd = wA.tile([P, BN], dt.float32, name="rstd",
                                   tag="rstd")
                    nc.scalar.activation(rstd[:], rv[:], Act.Sqrt)

                    cen = wA.tile([P, 2, BN], dt.bfloat16, name="cen",
                                  tag="cen")
                    nc.vector.tensor_add(
                        cen[:], h1[:],
                        mups[:, None, :].to_broadcast([P, 2, BN]))
                    cn = wA.tile([P, 2, BN], dt.bfloat16, name="cn", tag="cn")
                    nc.vector.tensor_mul(
                        cn[:], cen[:],
                        rstd[:, None, :].to_broadcast([P, 2, BN]))
                    hrelu = wA.tile([P, 2, BN], dt.bfloat16, name="hrelu",
                                    tag="hrelu")
                    for m in range(2):
                        nc.scalar.activation(
                            hrelu[:, m, :], cn[:, m, :], Act.Relu,
                            bias=lnb[:, m, 0:1], scale=lng[:, m, 0:1])

                    # node-major h2 + attention scalars -> table rows
                    asm = wA.tile([P, BN // P, ROW], dt.bfloat16, name="asm",
                                  tag="asm")
                    nc.gpsimd.memset(asm[:, :, 261:ROW], 0.0)
                    for t in range(BN // P):
                        tsl = slice(t * P, (t + 1) * P)
                        nmps = psA.tile([P, 261], dt.float32, name="nmps",
                                        tag="nmps")
                        for k in range(2):
                            nc.tensor.matmul(
                                nmps[:], lhsT=hrelu[:, k, tsl],
                                rhs=w2a[:, k, :],
                                start=(k == 0), stop=(k == 1))
                        nc.vector.tensor_add(asm[:, t, 0:261], nmps[:],
                                             b2rep[:])
                        if n0 + t * P < TPC:
                            w = (n0 + t * P) // P
                            nc.vector.tensor_copy(adstw[:, w, :],
                                                  asm[:, t, 259:261])

                    # feature-major h2 for the skip connection (targets only)
                    if n0 < TPC:
                        for m in range(2):
                            fmps = psA.tile([P, BN], dt.float32, name="fmps",
                                            tag="nmps")
                            for k in range(2):
                                nc.tensor.matmul(
                                    fmps[:],
                                    lhsT=w2a[:, k, m * P : (m + 1) * P],
                                    rhs=hrelu[:, k, :],
                                    start=(k == 0), stop=(k == 1))
                            nc.scalar.activation(
                                h2loc[:, m, n0 : n0 + BN], fmps[:],
                                Act.Identity, bias=b2c[:, m, 0:1])
                    dst = tab_w[:].rearrange("(bb tt pp) r -> bb pp tt r",
                                             bb=NBLK, pp=P)[b]
                    nc.sync.dma_start(out=dst, in_=asm[:])

            if sharded:
                nc.gpsimd.collective_compute(
                    "AllGather",
                    mybir.AluOpType.bypass,
                    replica_groups=[list(range(NCORES))],
                    ins=[cc_in.opt()],
                    outs=[cc_out.opt()],
                )

            # ================= Phase B: edge aggregation ==================
            win_t0 = []
            t0 = 0
            for w in range(WPC):
                win_t0.append(t0)
                t0 += tiles_per_win[w]

            def _chunks(base, n):
                k = math.ceil(n / GH)
                sizes = [n // k + (1 if i < n % k else 0) for i in range(k)]
                out, b0 = [], base
                for s in sizes:
                    out.append((b0, s))
                    b0 += s
                return out

            with (
                tc.tile_pool(name="wB", bufs=3) as wB,
                tc.tile_pool(name="gpool", bufs=3) as gpool,
                tc.tile_pool(name="psB", bufs=2, space="PSUM") as psB,
                tc.tile_pool(name="psBd", bufs=2, space="PSUM") as psBd,
            ):
                for w in range(WPC):
                    ntw = tiles_per_win[w]
                    halves = _chunks(win_t0[w], ntw)
                    agg0 = psB.tile([P, 257], dt.float32, name="agg0",
                                    tag="agg0")
                    agg1 = psB.tile([P, 257], dt.float32, name="agg1",
                                    tag="agg1")
                    aggh = [agg0, agg1]
                    done = 0
                    for hb, hn in halves:
                        gb = gpool.tile([P, GH, ROW], dt.bfloat16, name="gb",
                                        tag="gb")
                        nc.gpsimd.dma_gather(
                            out_ap=gb[:, :hn, :],
                            in_ap=tab_r[:],
                            idxs_ap=gidx_s[:, hb * 8 : (hb + hn) * 8],
                            num_idxs=hn * P,
                            num_idxs_reg=hn * P,
                            elem_size=ROW,
                        )
                        of = wB.tile([P, GH, P], dt.bfloat16, name="ohf",
                                     tag="ohf")
                        nc.sync.dma_start(out=of[:, :hn, :],
                                          in_=ohT_in[:, hb : hb + hn, :])

                        dps = psBd.tile([P, 2 * GH], dt.float32, name="dps",
                                        tag="dps")
                        for i in range(hn):
                            nc.tensor.matmul(
                                dps[:, 2 * i : 2 * i + 2],
                                lhsT=of[:, i, :],
                                rhs=adstw[:, w, :],
                                start=(i == 0), stop=(i == hn - 1),
                                skip_group_check=True)
                        esb = wB.tile([P, GH, 2], dt.float32, name="esb",
                                      tag="esb")
                        nc.vector.tensor_add(
                            esb[:, :hn, :],
                            gb[:, :hn, 257:259],
                            dps[:, : 2 * hn].rearrange(
                                "p (t two) -> p t two", two=2))
                        lk = wB.tile([P, GH, 2], dt.float32, name="lk",
                                     tag="lk")
                        nc.vector.scalar_tensor_tensor(
                            lk[:, :hn, :], esb[:, :hn, :], 0.2,
                            esb[:, :hn, :], op0=Alu.mult, op1=Alu.max)
                        wexp = wB.tile([P, GH, 2], dt.bfloat16, name="wexp",
                                       tag="wexp")
                        nc.scalar.activation(wexp[:, :hn, :], lk[:, :hn, :],
                                             Act.Exp)

                        for i in range(hn):
                            t = hb + i
                            ohw = wB.tile([P, 2, P], dt.bfloat16, name="ohw",
                                          tag="ohw", bufs=4)
                            nc.vector.scalar_tensor_tensor(
                                ohw[:], iota2[:], meta_s[:, t : t + 1],
                                wexp[:, i, :, None].to_broadcast([P, 2, P]),
                                op0=Alu.is_equal, op1=Alu.mult)
                            for h in range(HEADS):
                                nc.tensor.matmul(
                                    aggh[h][:],
                                    lhsT=ohw[:, h, :],
                                    rhs=gb[:, i, 0:257],
                                    start=(done == 0),
                                    stop=(done == ntw - 1),
                                    skip_group_check=True)
                            done += 1
                    for h in range(HEADS):
                        rz = wB.tile([P, 1], dt.float32, name="rz", tag="rz")
                        nc.vector.reciprocal(rz[:], aggh[h][:, 256:257])
                        nc.scalar.activation(
                            aggs[:, w, h, :], aggh[h][:, 0:HOUT],
                            Act.Identity, scale=rz[:])

            # ================= Phase C: epilogue ==========================
            with (
                tc.tile_pool(name="wC", bufs=1) as wC,
                tc.tile_pool(name="wC2", bufs=2) as wC2,
                tc.tile_pool(name="psC", bufs=2, space="PSUM") as psC,
                tc.tile_pool(name="psC2", bufs=2, space="PSUM") as psC2,
            ):
                # transpose aggs -> k-major for the transform GEMM
                agg2T = wC.tile([P, 2, 2, TPC], dt.bfloat16, name="agg2T",
                                tag="agg2T")
                for w in range(WPC):
                    for h in range(HEADS):
                        for kc in range(2):
                            tp = psC.tile([P, P], dt.bfloat16, name="tpC",
                                          tag="tpC")
                            nc.tensor.transpose(
                                tp[:], aggs[:, w, h, kc * P : (kc + 1) * P],
                                ident[:])
                            dsl = agg2T[:, h, kc, w * P : (w + 1) * P]
                            if (h + kc) % 2:
                                nc.scalar.copy(dsl, tp[:])
                            else:
                                nc.vector.tensor_copy(dsl, tp[:])

                # conv (W_g) + skip GEMMs accumulate in one PSUM group; ELU
                outT = wC.tile([P, 4, TPC], dt.bfloat16, name="outT",
                               tag="outT")
                for fc in range(4):
                    h = fc // 2
                    fsl = slice(fc * P, (fc + 1) * P)
                    for n in range(2):
                        nsl = slice(n * 512, (n + 1) * 512)
                        cs = psC2.tile([P, 512], dt.float32, name="csps",
                                       tag="csps")
                        for kc in range(2):
                            nc.tensor.matmul(
                                cs[:], lhsT=gats[:, kc, fsl],
                                rhs=agg2T[:, h, kc, nsl],
                                start=(kc == 0), stop=False,
                                skip_group_check=True)
                        for k in range(2):
                            nc.tensor.matmul(
                                cs[:], lhsT=skips[:, k, fsl],
                                rhs=h2loc[:, k, nsl],
                                start=False, stop=(k == 1),
                                skip_group_check=True)
                        t_sb = wC2.tile([P, 512], dt.bfloat16, name="t_sb",
                                        tag="t_sb")
                        if n == 0:
                            nc.scalar.activation(t_sb[:], cs[:], Act.Identity,
                                                 bias=gbsk[:, fc, 0:1])
                        else:
                            nc.vector.tensor_scalar_add(t_sb[:], cs[:],
                                                        gbsk[:, fc, 0:1])
                        mn = wC2.tile([P, 512], dt.bfloat16, name="mn",
                                      tag="mn")
                        nc.vector.tensor_scalar_min(mn[:], t_sb[:], 0.0)
                        ez = wC2.tile([P, 512], dt.bfloat16, name="ez",
                                      tag="ez")
                        nc.scalar.activation(ez[:], mn[:], Act.Exp,
                                             bias=ln01[:, 0:1])
                        rl = wC2.tile([P, 512], dt.bfloat16, name="rl",
                                      tag="rl")
                        nc.vector.tensor_scalar_max(rl[:], t_sb[:], 0.0)
                        nc.vector.scalar_tensor_tensor(
                            outT[:, fc, nsl], ez[:], -0.1, rl[:],
                            op0=Alu.add, op1=Alu.add)

                # decoder
                dsb = wC.tile([P, 8, TPC], dt.bfloat16, name="dsb", tag="dsb")
                for m in range(8):
                    for n in range(2):
                        nsl = slice(n * 512, (n + 1) * 512)
                        ps = psC2.tile([P, 512], dt.float32, name="decps",
                                       tag="decps")
                        for k in range(4):
                            nc.tensor.matmul(
                                ps[:], lhsT=d1s[:, k, m * P : (m + 1) * P],
                                rhs=outT[:, k, nsl],
                                start=(k == 0), stop=(k == 3))
                        tmp = wC2.tile([P, 512], dt.float32, name="dtmp",
                                       tag="dtmp")
                        if (m + n) % 2:
                            nc.scalar.activation(tmp[:], ps[:], Act.Identity,
                                                 bias=db1s[:, m, 0:1])
                        else:
                            nc.vector.tensor_scalar_add(tmp[:], ps[:],
                                                        db1s[:, m, 0:1])
                        nc.vector.scalar_tensor_tensor(
                            dsb[:, m, nsl], tmp[:], 0.1, tmp[:],
                            op0=Alu.mult, op1=Alu.max)

                ysb = wC.tile([1, TPC], dt.float32, name="ysb", tag="ysb")
                for n in range(2):
                    nsl = slice(n * 512, (n + 1) * 512)
                    yp = psC.tile([1, 512], dt.float32, name="yps", tag="yps")
                    for m in range(8):
                        nc.tensor.matmul(
                            yp[:], lhsT=d2s[:, m, 0:1],
                            rhs=dsb[:, m, nsl],
                            start=(m == 0), stop=(m == 7))
                    nc.scalar.activation(ysb[:, nsl], yp[:], Act.Identity,
                                         bias=db2s[0:1, 0:1])
                nc.sync.dma_start(out=y_out[:], in_=ysb[:])

    nc.compile()
    return nc


# ----------------------------------------------------------------------------
# Driver
# ----------------------------------------------------------------------------

def _host_in_maps(inputs, buckets, tiles_per_win, sharded=SHARDED):
    x = np.asarray(inputs["x"], dtype=F32)
    enc_w1 = np.asarray(inputs["enc_w1"], F32)
    enc_b1 = np.asarray(inputs["enc_b1"], F32)
    ln_g = np.asarray(inputs["ln_g"], F32)
    ln_b = np.asarray(inputs["ln_b"], F32)
    enc_w2 = np.asarray(inputs["enc_w2"], F32)
    enc_b2 = np.asarray(inputs["enc_b2"], F32)
    gat_w = np.asarray(inputs["gat_w"], F32)
    att_src = np.asarray(inputs["att_src"], F32).reshape(HEADS, HOUT)
    att_dst = np.asarray(inputs["att_dst"], F32).reshape(HEADS, HOUT)
    gat_bias = np.asarray(inputs["gat_bias"], F32)
    skip_w = np.asarray(inputs["skip_w"], F32)
    skip_b = np.asarray(inputs["skip_b"], F32)
    dec_w1 = np.asarray(inputs["dec_w1"], F32)
    dec_b1 = np.asarray(inputs["dec_b1"], F32)
    dec_w2 = np.asarray(inputs["dec_w2"], F32)
    dec_b2 = np.asarray(inputs["dec_b2"], F32)

    # attention vectors folded to h2 space: att4[f2, c] for c in
    # [src_h0, src_h1, dst_h0, dst_h1]
    att4 = np.zeros((H, 4), dtype=F32)
    for h in range(HEADS):
        blk = gat_w[h * HOUT : (h + 1) * HOUT, :]
        att4[:, h] = blk.T @ att_src[h]
        att4[:, 2 + h] = blk.T @ att_dst[h]

    w2T = np.ascontiguousarray(enc_w2.T)             # [f1, f2]
    attW = w2T @ att4                                # [f1, 4]
    w2a = np.zeros((H, 261), dtype=F32)
    w2a[:, 0:256] = w2T
    w2a[:, 257:261] = attW
    b2rep_row = np.zeros((261,), dtype=F32)
    b2rep_row[0:256] = enc_b2
    b2rep_row[256] = 1.0
    b2rep_row[257:261] = enc_b2 @ att4

    negpos = np.zeros((P, 2, P), dtype=F32)
    negpos[:, 0, :] = -1.0 / H
    negpos[:, 1, :] = 1.0 / H
    iota2 = np.tile(np.arange(P, dtype=F32), (P, 2, 1))

    ntiles = sum(tiles_per_win)

    def kc(a, k):
        """[k*128, cols] -> per-partition packed [P, k*cols]."""
        a = np.asarray(a, F32)
        cols = a.shape[1]
        return a.reshape(k, P, cols).transpose(1, 0, 2).reshape(P, k * cols)

    def rep(v):
        """[n] vector -> [P, n] replicated."""
        return np.tile(np.asarray(v, F32).reshape(1, -1), (P, 1))

    bf_parts = [
        kc(enc_w1.T, 1),                      # w1s [P, 256]
        negpos.reshape(P, 2 * P),             # negpos
        kc(w2a, 2),                           # w2a [P, 522]
        kc(gat_w.T, 2),                       # gats [P, 1024]
        kc(skip_w.T, 2),                      # skips [P, 1024]
        kc(dec_w1.T, 4),                      # d1s [P, 4096]
        kc(dec_w2.T, 8),                      # d2s [P, 8]
        iota2.reshape(P, 2 * P),              # iota2
        np.eye(P, dtype=F32),                 # ident
    ]
    f32_parts = [
        np.tile(b2rep_row, (P, 1)),           # b2rep [P, 261]
        kc(enc_b2.reshape(-1, 1), 2),         # b2c [P, 2]
        kc(enc_b1.reshape(-1, 1), 2),         # b1 [P, 2]
        kc(ln_g.reshape(-1, 1), 2),           # lng
        kc(ln_b.reshape(-1, 1), 2),           # lnb
        kc((gat_bias + skip_b).reshape(-1, 1), 4),   # gbsk [P, 4]
        kc(dec_b1.reshape(-1, 1), 8),         # db1 [P, 8]
        rep(dec_b2),                          # db2 [P, 1]
        np.full((P, 1), np.log(0.1), F32),    # ln01
    ]
    cf32 = np.ascontiguousarray(
        np.concatenate(f32_parts, axis=1).astype(F32))

    pos = _table_pos()
    in_maps = []
    for c in range(NCORES):
        wrapped, meta, ohT = _per_core_arrays(buckets, tiles_per_win, c, pos)
        cbf = np.ascontiguousarray(
            np.concatenate(
                bf_parts + [meta.astype(F32)], axis=1).astype(BF16))
        rows = _core_rows(c)
        xp = np.zeros((SHARD, N_IN), dtype=F32)
        valid = rows < N_NODES
        xp[valid] = x[rows[valid]]
        m = {
            "xT": np.ascontiguousarray(xp.T.astype(BF16)),
            "cbf": cbf,
            "cf32": cf32,
            "gidx": wrapped,
            "ohT": ohT,
        }
        in_maps.append(m)
    return in_maps


def prepared(inputs):
    edge_index = np.asarray(inputs["edge_index"])
    buckets, tiles_per_win = _prepare_edges(edge_index)
    key = tuple(tiles_per_win)
    if key not in _cache:
        _cache[key] = _build_program(tiles_per_win)
    nc = _cache[key]
    in_maps = _host_in_maps(inputs, buckets, tiles_per_win)
    return nc, in_maps


def kernel(x, edge_index, batch_size, enc_w1, enc_b1, ln_g, ln_b, enc_w2,
           enc_b2, gat_w, att_src, att_dst, gat_bias, skip_w, skip_b,
           dec_w1, dec_b1, dec_w2, dec_b2, _trace=False):
    inputs = dict(x=x, edge_index=edge_index, enc_w1=enc_w1, enc_b1=enc_b1,
                  ln_g=ln_g, ln_b=ln_b, enc_w2=enc_w2, enc_b2=enc_b2,
                  gat_w=gat_w, att_src=att_src, att_dst=att_dst,
                  gat_bias=gat_bias, skip_w=skip_w, skip_b=skip_b,
                  dec_w1=dec_w1, dec_b1=dec_b1, dec_w2=dec_w2, dec_b2=dec_b2)
    nc, in_maps = prepared(inputs)

    from concourse.bass_utils import run_bass_kernel_spmd
    res = run_bass_kernel_spmd(
        nc, in_maps, core_ids=list(range(NCORES)), trace=_trace)

    y = np.concatenate([res.results[c]["y"][0] for c in range(NCORES)])
    out = y.reshape(BATCH, 1).astype(F32)
    if _trace:
        return out, res
    return out
